# revision 3
# baseline (speedup 1.0000x reference)
"""Single-head causal attention (B=4, T=4096, C=768, H=64) on 8 NeuronCores.

Sharding: 2 cores per batch; keys split between the pair by interleaved
128-row blocks.  Host permutes token columns per core (own/other groups of
256) so the device program is parity-independent; host adds the two cores'
unnormalized partials and normalizes.

The Tile framework preserves per-engine emission order, so this version is
explicitly software-pipelined with precise-dependency filler queues:
 - kv-block matmuls ("kvN") and the v'-tile builds ("kvNv") are separate
   queues; q-projections ("qN") likewise.  A queue is force-drained right
   before the first instruction that needs its result; otherwise units are
   popped between score pairs as PE filler.
 - each tile's diagonal pair is computed FIRST (its Pool mask-multiply gets
   a whole tile of slack) and its PV + output copy are DEFERRED into the
   next tile.

Math per core (bf16 projections/scores; fp8 softmax weights; v split into
hi+lo fp8, PV via fp8 DoubleRow matmuls; denominator via ones-column).
"""

import sys

for _p in ("/opt/trn_rl_repo",):
    if _p not in sys.path:
        sys.path.insert(0, _p)

import math
from collections import OrderedDict, deque
import numpy as np
import ml_dtypes

import concourse.bass as bass
import concourse.mybir as mybir
import concourse.tile as tile
from concourse import bacc
from concourse import bass_utils
from concourse.masks import make_identity

BF16 = mybir.dt.bfloat16
F8 = mybir.dt.float8e4
F32 = mybir.dt.float32

P = 128
T = 4096
C = 768
H = 64
CC = C // P
OWN = T // 2
NJ = T // 512
NCORES = 8
LN8 = math.log(8.0)
NWMK = CC * 64 + CC * 128 + 1024

_NC_CACHE = {}


def _build_nc():
    nc = bacc.Bacc("TRN2", target_bir_lowering=False, debug=False,
                   num_devices=NCORES)

    xT = nc.dram_tensor("xT", [C, T], BF16, kind="ExternalInput")
    wmk = nc.dram_tensor("wmk", [P, NWMK], BF16, kind="ExternalInput")
    outp = nc.dram_tensor("outp", [NJ, 65, 512], F32, kind="ExternalOutput")

    with tile.TileContext(nc) as tc:
        with (
            tc.tile_pool(name="const", bufs=1) as cst,
            tc.tile_pool(name="big", bufs=1) as big,
            tc.tile_pool(name="pps", bufs=1, space="PSUM") as pps,
            tc.tile_pool(name="sps", bufs=2, space="PSUM") as sps_pool,
            tc.tile_pool(name="ops", bufs=1, space="PSUM") as ops_pool,
            tc.tile_pool(name="qpp", bufs=1, space="PSUM") as qpp,
            tc.tile_pool(name="wt", bufs=6) as wt_pool,
            tc.tile_pool(name="osb", bufs=4) as osb_pool,
        ):
            bias_t = cst.tile([P, 1], F32)
            nc.vector.memset(bias_t[:], -LN8)
            warm = cst.tile([P, 1], F8)
            nc.scalar.activation(warm[:], bias_t[:],
                                 mybir.ActivationFunctionType.Exp,
                                 scale=0.125, bias=bias_t[:])
            ident = cst.tile([P, P], BF16)
            make_identity(nc, ident[:])
            wmk_sb = cst.tile([P, NWMK], BF16)
            nc.sync.dma_start(wmk_sb[:, 0:CC * 192], wmk[:, 0:CC * 192])
            nc.sync.dma_start(wmk_sb[:, CC * 192:], wmk[:, CC * 192:])

            def wq_ap(ci):
                return wmk_sb[:, 64 * ci:64 * (ci + 1)]

            def wkv_ap(ci):
                return wmk_sb[:, CC * 64 + 128 * ci:CC * 64 + 128 * (ci + 1)]

            msk_ap = wmk_sb[:, CC * 192:CC * 192 + 1024]

            xts = []
            for ci in range(CC):
                t_ = big.tile([P, T], BF16, tag=f"xt{ci}")
                xts.append(t_)
            pieces = [(0, 1024), (1024, 2048), (2048, 3072), (3072, 4096)]
            for lo, hi in pieces:
                for ci in range(CC):
                    nc.sync.dma_start(xts[ci][:, lo:hi],
                                      xT[P * ci:P * (ci + 1), lo:hi])

            vhi, vlo = [], []
            for pb in range(NJ):
                th = big.tile([P, 256], F8, tag=f"vhi{pb}")
                nc.gpsimd.memset(th[:], 0.0)
                nc.gpsimd.memset(th[:, 64:65], 1.0)
                nc.gpsimd.memset(th[:, 192:193], 1.0)
                vhi.append(th)
                tl = big.tile([P, 256], F8, tag=f"vlo{pb}")
                nc.gpsimd.memset(tl[:], 0.0)
                vlo.append(tl)

            kvts = [big.tile([P, 512], BF16, tag=f"kvT{tb}",
                             name=f"kvT{tb}") for tb in range(4)]
            qts = [big.tile([64, 512], BF16, tag=f"qT{j}",
                            name=f"qT{j}") for j in range(NJ)]

            # ---- precise-dependency filler queues ----
            queues = OrderedDict()   # name -> deque of closures

            def q_add(name, fn):
                queues.setdefault(name, deque()).append(fn)

            def require(name):
                q = queues.get(name)
                if not q:
                    return
                while q:
                    q.popleft()()

            def pop_filler(n):
                for _ in range(n):
                    for name, q in queues.items():
                        if q:
                            q.popleft()()
                            break
                    else:
                        return

            def queue_kv(tb):
                ps_box = {}

                def mm(ci):
                    def f():
                        if ci == 0:
                            ps_box["ps"] = pps.tile([P, 512], F32,
                                                    tag="pps", name="ps")
                        src = xts[ci][:].rearrange("p (g c) -> p g c", c=256)
                        nc.tensor.matmul(
                            ps_box["ps"][:], wkv_ap(ci),
                            src[:, 4 * tb:4 * tb + 4:2, :],
                            start=(ci == 0), stop=(ci == CC - 1))
                    return f

                def kvt_copy():
                    nc.vector.tensor_copy(kvts[tb][:], ps_box["ps"][:])

                for ci in range(CC):
                    q_add(f"kv{tb}", mm(ci))
                q_add(f"kv{tb}", kvt_copy)

                def transp(i):
                    def f():
                        if i == 0:
                            ps_box["vp"] = pps.tile([P, 256], BF16,
                                                    tag="vp", name="vp")
                        nc.tensor.transpose(
                            ps_box["vp"][:, 64 * i:64 * (i + 1)],
                            kvts[tb][64:128, 128 * i:128 * (i + 1)],
                            ident[64:128, 64:128])
                    return f

                def vcopy(i):
                    def f():
                        g = 4 * tb + i
                        pb, s = g // 2, g % 2
                        dst_h = vhi[pb][:, 128 * s:128 * s + 64]
                        dst_l = vlo[pb][:, 128 * s:128 * s + 64]
                        vp = ps_box["vp"]
                        nc.vector.tensor_copy(dst_h, vp[:, 64 * i:64 * (i + 1)])
                        nc.vector.tensor_sub(dst_l, vp[:, 64 * i:64 * (i + 1)],
                                             dst_h)
                    return f

                for i in range(4):
                    q_add(f"kv{tb}v", transp(i))
                for i in range(4):
                    q_add(f"kv{tb}v", vcopy(i))

            def queue_q(j):
                qp_box = {}

                def mm(ci):
                    def f():
                        if ci == 0:
                            qp_box["qp"] = qpp.tile([64, 512], F32,
                                                    tag="qp", name="qp")
                        nc.tensor.matmul(
                            qp_box["qp"][:], wq_ap(ci),
                            xts[ci][:, 512 * j:512 * (j + 1)],
                            start=(ci == 0), stop=(ci == CC - 1))
                    return f

                def qt_copy():
                    nc.vector.tensor_copy(qts[j][:], qp_box["qp"][:])

                for ci in range(CC):
                    q_add(f"q{j}", mm(ci))
                q_add(f"q{j}", qt_copy)

            def scores_pair(sp, pb, j):
                for i in range(2):
                    g = 2 * pb + i
                    nc.tensor.matmul(
                        sp[:, 512 * i:512 * (i + 1)],
                        kvts[g // 4][0:64, 128 * (g % 4):128 * (g % 4 + 1)],
                        qts[j][:], start=True, stop=True)

            def dr_pair(ops, pb, wt, start, stop):
                require(f"kv{pb // 2}v")
                wt3 = wt[:].rearrange("p (two n) -> p two n", two=2)
                nc.tensor.matmul(
                    ops[:], vhi[pb][:].rearrange("p (two m) -> p two m", two=2),
                    wt3, start=start, stop=False,
                    perf_mode=mybir.MatmulPerfMode.DoubleRow)
                nc.tensor.matmul(
                    ops[:], vlo[pb][:].rearrange("p (two m) -> p two m", two=2),
                    wt3, start=False, stop=stop,
                    perf_mode=mybir.MatmulPerfMode.DoubleRow)

            # pending diagonal PV from the previous tile: (wtd, j, ops, start)
            state = {"diag": None}

            def flush_diag():
                pend = state["diag"]
                if pend is None:
                    return
                wtd, jj, ops_prev, start = pend
                state["diag"] = None
                dr_pair(ops_prev, jj, wtd, start, True)
                osb = osb_pool.tile([65, 512], F32, tag="osb")
                nc.vector.tensor_copy(osb[:], ops_prev[0:65, :])
                nc.sync.dma_start(outp[jj], osb[:])

            # queue all projection work up front (popped as filler in order;
            # force-drained by require() right before first use)
            queue_kv(0)
            queue_q(0)
            queue_q(1)

            for j in range(NJ):
                if j % 2 == 1 and (j + 1) // 2 < 4:
                    queue_kv((j + 1) // 2)
                if j + 2 < NJ:
                    queue_q(j + 2)

                require(f"kv{(2 * j + 1) // 4}")
                require(f"q{j}")

                ops = ops_pool.tile([P, 512], F32, tag="ops")
                # diagonal pair first; the odd chunk only needs query cols
                # {128:256, 384:512} (union of both parities' unmasked cols)
                spd = sps_pool.tile([P, 1024], F32, tag="sps")
                g0 = 2 * j
                nc.tensor.matmul(
                    spd[:, 0:512],
                    kvts[g0 // 4][0:64, 128 * (g0 % 4):128 * (g0 % 4 + 1)],
                    qts[j][:], start=True, stop=True)
                g1 = 2 * j + 1
                kslc = kvts[g1 // 4][0:64, 128 * (g1 % 4):128 * (g1 % 4 + 1)]
                nc.tensor.matmul(spd[:, 640:768], kslc, qts[j][:, 128:256],
                                 start=True, stop=True)
                nc.tensor.matmul(spd[:, 896:1024], kslc, qts[j][:, 384:512],
                                 start=True, stop=True)
                wtd = wt_pool.tile([P, 1024], F8, tag="wtd")
                nc.scalar.activation(
                    wtd[:, 0:512], spd[:, 0:512],
                    mybir.ActivationFunctionType.Exp,
                    scale=0.125, bias=bias_t[:])
                spd3 = spd[:].rearrange("p (g c) -> p g c", c=128)
                wtd3 = wtd[:].rearrange("p (g c) -> p g c", c=128)
                nc.scalar.activation(
                    wtd3[:, 5:8:2, :], spd3[:, 5:8:2, :],
                    mybir.ActivationFunctionType.Exp,
                    scale=0.125, bias=bias_t[:])
                nc.gpsimd.memset(wtd3[:, 4:7:2, :], 0.0)
                nc.gpsimd.tensor_mul(wtd[:], wtd[:], msk_ap)
                pop_filler(2)

                prev = None
                for pb in range(j):
                    sp = sps_pool.tile([P, 1024], F32, tag="sps")
                    scores_pair(sp, pb, j)
                    wt = wt_pool.tile([P, 1024], F8, tag="wt")
                    nc.scalar.activation(
                        wt[:], sp[:], mybir.ActivationFunctionType.Exp,
                        scale=0.125, bias=bias_t[:])
                    if pb == 0:
                        flush_diag()     # previous tile's diagonal PV + out
                    if prev is not None:
                        dr_pair(ops, prev[1], prev[0], prev[1] == 0, False)
                    prev = (wt, pb)
                    pop_filler(3)
                if prev is not None:
                    dr_pair(ops, prev[1], prev[0], prev[1] == 0, False)
                flush_diag()             # no-op except after tile 0
                state["diag"] = (wtd, j, ops, j == 0)
            flush_diag()

    nc.compile()
    return nc


def get_nc():
    if "nc" not in _NC_CACHE:
        _NC_CACHE["nc"] = _build_nc()
    return _NC_CACHE["nc"]


def _perm_for_parity(p):
    t = np.arange(T)
    own = t[(t // 128) % 2 == p]
    oth = t[(t // 128) % 2 != p]
    perm = np.empty(T, np.int64)
    pos = np.arange(OWN)
    perm[512 * (pos // 256) + pos % 256] = own
    perm[512 * (pos // 256) + 256 + pos % 256] = oth
    return perm


def _mask_for_parity(p):
    si = np.arange(P)[:, None]
    ti = np.arange(512)[None, :]
    rel_q = np.where(
        ti < 256,
        256 * (ti // 128) + 128 * p + ti % 128,
        256 * ((ti - 256) // 128) + 128 * (1 - p) + (ti - 256) % 128)
    m = np.empty((P, 1024), np.float32)
    for e in range(2):
        key_rel = 256 * e + 128 * p + si
        m[:, 512 * e:512 * (e + 1)] = np.where(key_rel <= rel_q, 1.0, 0.0)
    return m


def make_in_maps(x, Wq, Wk, Wv):
    bf = ml_dtypes.bfloat16
    wq_in = np.zeros((P, CC * 64), np.float32)
    wkv_in = np.zeros((P, CC * 128), np.float32)
    for ci in range(CC):
        wq_in[:, 64 * ci:64 * (ci + 1)] = Wq[P * ci:P * (ci + 1), :]
        wkv_in[:, 128 * ci:128 * ci + 64] = Wk[P * ci:P * (ci + 1), :]
        wkv_in[:, 128 * ci + 64:128 * (ci + 1)] = Wv[P * ci:P * (ci + 1), :]
    in_maps = []
    perms = [_perm_for_parity(0), _perm_for_parity(1)]
    msks = [_mask_for_parity(0), _mask_for_parity(1)]
    for c in range(NCORES):
        b, p = c // 2, c % 2
        xb = np.asarray(x[b], dtype=np.float32)
        xT_in = np.ascontiguousarray(xb[perms[p]].T).astype(bf)
        wmk_in = np.concatenate([wq_in, wkv_in, msks[p]], axis=1).astype(bf)
        in_maps.append({"xT": xT_in, "wmk": wmk_in})
    return in_maps


def combine(results, B=4):
    perms = [_perm_for_parity(0), _perm_for_parity(1)]
    out = np.zeros((B, T, H), np.float32)
    num = np.zeros((T, H), np.float32)
    den = np.zeros((T,), np.float32)
    for b in range(B):
        num[:] = 0.0
        den[:] = 0.0
        for p in range(2):
            o = results[2 * b + p]["outp"].astype(np.float32)
            cols = perms[p].reshape(NJ, 512)
            for j in range(NJ):
                num[cols[j]] += o[j, :64, :].T
                den[cols[j]] += o[j, 64, :]
        out[b] = num / den[:, None]
    return out


def kernel(x, Wq, Wk, Wv, **run_kwargs):
    nc = get_nc()
    in_maps = make_in_maps(x, Wq, Wk, Wv)
    res = bass_utils.run_bass_kernel_spmd(nc, in_maps,
                                          list(range(NCORES)), **run_kwargs)
    out = combine(res.results, B=x.shape[0])
    if run_kwargs:
        kernel.last_results = res
    return out


# revision 4
# speedup vs baseline: 1.0069x; 1.0069x over previous
"""Single-head causal attention (B=4, T=4096, C=768, H=64) on 8 NeuronCores.

Sharding: 2 cores per batch; keys split between the pair by interleaved
128-row blocks.  Host permutes token columns per core (own/other groups of
256) so the device program is parity-independent; host adds the two cores'
unnormalized partials and normalizes.

The Tile framework preserves per-engine emission order, so this version is
explicitly software-pipelined with precise-dependency filler queues:
 - kv-block matmuls ("kvN") and the v'-tile builds ("kvNv") are separate
   queues; q-projections ("qN") likewise.  A queue is force-drained right
   before the first instruction that needs its result; otherwise units are
   popped between score pairs as PE filler.
 - each tile's diagonal pair is computed FIRST (its Pool mask-multiply gets
   a whole tile of slack) and its PV + output copy are DEFERRED into the
   next tile.

Math per core (bf16 projections/scores; fp8 softmax weights; v split into
hi+lo fp8, PV via fp8 DoubleRow matmuls; denominator via ones-column).
"""

import sys

for _p in ("/opt/trn_rl_repo",):
    if _p not in sys.path:
        sys.path.insert(0, _p)

import math
from collections import OrderedDict, deque
import numpy as np
import ml_dtypes

import concourse.bass as bass
import concourse.mybir as mybir
import concourse.tile as tile
from concourse import bacc
from concourse import bass_utils
from concourse.masks import make_identity

BF16 = mybir.dt.bfloat16
F8 = mybir.dt.float8e4
F32 = mybir.dt.float32

P = 128
T = 4096
C = 768
H = 64
CC = C // P
OWN = T // 2
NJ = T // 512
NCORES = 8
LN8 = math.log(8.0)
NWMK = CC * 64 + CC * 128 + 1024

_NC_CACHE = {}


def _build_nc():
    nc = bacc.Bacc("TRN2", target_bir_lowering=False, debug=False,
                   num_devices=NCORES)

    xT = nc.dram_tensor("xT", [C, T], BF16, kind="ExternalInput")
    wmk = nc.dram_tensor("wmk", [P, NWMK], BF16, kind="ExternalInput")
    outp = nc.dram_tensor("outp", [NJ, 65, 512], F32, kind="ExternalOutput")

    with tile.TileContext(nc) as tc:
        with (
            tc.tile_pool(name="const", bufs=1) as cst,
            tc.tile_pool(name="big", bufs=1) as big,
            tc.tile_pool(name="pps", bufs=1, space="PSUM") as pps,
            tc.tile_pool(name="sps", bufs=2, space="PSUM") as sps_pool,
            tc.tile_pool(name="ops", bufs=1, space="PSUM") as ops_pool,
            tc.tile_pool(name="qpp", bufs=1, space="PSUM") as qpp,
            tc.tile_pool(name="wt", bufs=8) as wt_pool,
            tc.tile_pool(name="osb", bufs=4) as osb_pool,
        ):
            bias_t = cst.tile([P, 1], F32)
            nc.vector.memset(bias_t[:], -LN8)
            warm = cst.tile([P, 1], F8)
            nc.scalar.activation(warm[:], bias_t[:],
                                 mybir.ActivationFunctionType.Exp,
                                 scale=0.125, bias=bias_t[:])
            ident = cst.tile([P, P], BF16)
            make_identity(nc, ident[:])
            wmk_sb = cst.tile([P, NWMK], BF16)
            nc.sync.dma_start(wmk_sb[:, 0:CC * 192], wmk[:, 0:CC * 192])
            nc.sync.dma_start(wmk_sb[:, CC * 192:], wmk[:, CC * 192:])

            def wq_ap(ci):
                return wmk_sb[:, 64 * ci:64 * (ci + 1)]

            def wkv_ap(ci):
                return wmk_sb[:, CC * 64 + 128 * ci:CC * 64 + 128 * (ci + 1)]

            msk_ap = wmk_sb[:, CC * 192:CC * 192 + 1024]

            xts = []
            for ci in range(CC):
                t_ = big.tile([P, T], BF16, tag=f"xt{ci}")
                xts.append(t_)
            pieces = [(0, 1024), (1024, 2048), (2048, 3072), (3072, 4096)]
            for lo, hi in pieces:
                for ci in range(CC):
                    nc.sync.dma_start(xts[ci][:, lo:hi],
                                      xT[P * ci:P * (ci + 1), lo:hi])

            vhi, vlo = [], []
            for pb in range(NJ):
                th = big.tile([P, 256], F8, tag=f"vhi{pb}")
                nc.gpsimd.memset(th[:], 0.0)
                nc.gpsimd.memset(th[:, 64:65], 1.0)
                nc.gpsimd.memset(th[:, 192:193], 1.0)
                vhi.append(th)
                tl = big.tile([P, 256], F8, tag=f"vlo{pb}")
                nc.gpsimd.memset(tl[:], 0.0)
                vlo.append(tl)

            kvts = [big.tile([P, 512], BF16, tag=f"kvT{tb}",
                             name=f"kvT{tb}") for tb in range(4)]
            qts = [big.tile([64, 512], BF16, tag=f"qT{j}",
                            name=f"qT{j}") for j in range(NJ)]

            # ---- precise-dependency filler queues ----
            queues = OrderedDict()   # name -> deque of closures

            def q_add(name, fn):
                queues.setdefault(name, deque()).append(fn)

            def require(name):
                q = queues.get(name)
                if not q:
                    return
                while q:
                    q.popleft()()

            def pop_filler(n):
                for _ in range(n):
                    for name, q in queues.items():
                        if q:
                            q.popleft()()
                            break
                    else:
                        return

            def queue_kv(tb):
                ps_box = {}

                def mm(ci):
                    def f():
                        if ci == 0:
                            ps_box["ps"] = pps.tile([P, 512], F32,
                                                    tag="pps", name="ps")
                        src = xts[ci][:].rearrange("p (g c) -> p g c", c=256)
                        nc.tensor.matmul(
                            ps_box["ps"][:], wkv_ap(ci),
                            src[:, 4 * tb:4 * tb + 4:2, :],
                            start=(ci == 0), stop=(ci == CC - 1))
                    return f

                def kvt_copy():
                    nc.vector.tensor_copy(kvts[tb][:], ps_box["ps"][:])

                for ci in range(CC):
                    q_add(f"kv{tb}", mm(ci))
                q_add(f"kv{tb}", kvt_copy)

                def transp(i):
                    def f():
                        if i == 0:
                            ps_box["vp"] = pps.tile([P, 256], BF16,
                                                    tag="vp", name="vp")
                        nc.tensor.transpose(
                            ps_box["vp"][:, 64 * i:64 * (i + 1)],
                            kvts[tb][64:128, 128 * i:128 * (i + 1)],
                            ident[64:128, 64:128])
                    return f

                def vcopy(i):
                    def f():
                        g = 4 * tb + i
                        pb, s = g // 2, g % 2
                        dst_h = vhi[pb][:, 128 * s:128 * s + 64]
                        dst_l = vlo[pb][:, 128 * s:128 * s + 64]
                        vp = ps_box["vp"]
                        nc.vector.tensor_copy(dst_h, vp[:, 64 * i:64 * (i + 1)])
                        nc.vector.tensor_sub(dst_l, vp[:, 64 * i:64 * (i + 1)],
                                             dst_h)
                    return f

                for i in range(4):
                    q_add(f"kv{tb}v", transp(i))
                for i in range(4):
                    q_add(f"kv{tb}v", vcopy(i))

            def queue_q(j):
                qp_box = {}

                def mm(ci):
                    def f():
                        if ci == 0:
                            qp_box["qp"] = qpp.tile([64, 512], F32,
                                                    tag="qp", name="qp")
                        nc.tensor.matmul(
                            qp_box["qp"][:], wq_ap(ci),
                            xts[ci][:, 512 * j:512 * (j + 1)],
                            start=(ci == 0), stop=(ci == CC - 1))
                    return f

                def qt_copy():
                    nc.vector.tensor_copy(qts[j][:], qp_box["qp"][:])

                for ci in range(CC):
                    q_add(f"q{j}", mm(ci))
                q_add(f"q{j}", qt_copy)

            def scores_pair(sp, pb, j):
                for i in range(2):
                    g = 2 * pb + i
                    nc.tensor.matmul(
                        sp[:, 512 * i:512 * (i + 1)],
                        kvts[g // 4][0:64, 128 * (g % 4):128 * (g % 4 + 1)],
                        qts[j][:], start=True, stop=True)

            def dr_pair(ops, pb, wt, start, stop):
                require(f"kv{pb // 2}v")
                wt3 = wt[:].rearrange("p (two n) -> p two n", two=2)
                nc.tensor.matmul(
                    ops[:], vhi[pb][:].rearrange("p (two m) -> p two m", two=2),
                    wt3, start=start, stop=False,
                    perf_mode=mybir.MatmulPerfMode.DoubleRow)
                nc.tensor.matmul(
                    ops[:], vlo[pb][:].rearrange("p (two m) -> p two m", two=2),
                    wt3, start=False, stop=stop,
                    perf_mode=mybir.MatmulPerfMode.DoubleRow)

            # pending diagonal PV from the previous tile: (wtd, j, ops, start)
            state = {"diag": None}

            def flush_diag():
                pend = state["diag"]
                if pend is None:
                    return
                wtd, jj, ops_prev, start = pend
                state["diag"] = None
                dr_pair(ops_prev, jj, wtd, start, True)
                osb = osb_pool.tile([65, 512], F32, tag="osb")
                nc.vector.tensor_copy(osb[:], ops_prev[0:65, :])
                nc.sync.dma_start(outp[jj], osb[:])

            # queue all projection work up front (popped as filler in order;
            # force-drained by require() right before first use)
            queue_kv(0)
            queue_q(0)
            queue_q(1)

            for j in range(NJ):
                if j % 2 == 1 and (j + 1) // 2 < 4:
                    queue_kv((j + 1) // 2)
                if j + 2 < NJ:
                    queue_q(j + 2)

                require(f"kv{(2 * j + 1) // 4}")
                require(f"q{j}")

                ops = ops_pool.tile([P, 512], F32, tag="ops")
                # diagonal pair first; the odd chunk only needs query cols
                # {128:256, 384:512} (union of both parities' unmasked cols)
                spd = sps_pool.tile([P, 1024], F32, tag="sps")
                g0 = 2 * j
                nc.tensor.matmul(
                    spd[:, 0:512],
                    kvts[g0 // 4][0:64, 128 * (g0 % 4):128 * (g0 % 4 + 1)],
                    qts[j][:], start=True, stop=True)
                g1 = 2 * j + 1
                kslc = kvts[g1 // 4][0:64, 128 * (g1 % 4):128 * (g1 % 4 + 1)]
                nc.tensor.matmul(spd[:, 640:768], kslc, qts[j][:, 128:256],
                                 start=True, stop=True)
                nc.tensor.matmul(spd[:, 896:1024], kslc, qts[j][:, 384:512],
                                 start=True, stop=True)
                wtd = wt_pool.tile([P, 1024], F8, tag="wtd")
                nc.scalar.activation(
                    wtd[:, 0:512], spd[:, 0:512],
                    mybir.ActivationFunctionType.Exp,
                    scale=0.125, bias=bias_t[:])
                spd3 = spd[:].rearrange("p (g c) -> p g c", c=128)
                wtd3 = wtd[:].rearrange("p (g c) -> p g c", c=128)
                nc.scalar.activation(
                    wtd3[:, 5:8:2, :], spd3[:, 5:8:2, :],
                    mybir.ActivationFunctionType.Exp,
                    scale=0.125, bias=bias_t[:])
                nc.gpsimd.memset(wtd3[:, 4:7:2, :], 0.0)
                nc.gpsimd.tensor_mul(wtd[:], wtd[:], msk_ap)
                pop_filler(2)

                pend = []
                for pb in range(j):
                    sp = sps_pool.tile([P, 1024], F32, tag="sps")
                    scores_pair(sp, pb, j)
                    wt = wt_pool.tile([P, 1024], F8, tag="wt")
                    nc.scalar.activation(
                        wt[:], sp[:], mybir.ActivationFunctionType.Exp,
                        scale=0.125, bias=bias_t[:])
                    if pb == 0:
                        flush_diag()     # previous tile's diagonal PV + out
                    pend.append((wt, pb))
                    if len(pend) > 2:
                        w0, p0 = pend.pop(0)
                        dr_pair(ops, p0, w0, p0 == 0, False)
                    pop_filler(3)
                for w0, p0 in pend:
                    dr_pair(ops, p0, w0, p0 == 0, False)
                pend = []
                flush_diag()             # no-op except after tile 0
                state["diag"] = (wtd, j, ops, j == 0)
            flush_diag()

    nc.compile()
    return nc


def get_nc():
    if "nc" not in _NC_CACHE:
        _NC_CACHE["nc"] = _build_nc()
    return _NC_CACHE["nc"]


def _perm_for_parity(p):
    t = np.arange(T)
    own = t[(t // 128) % 2 == p]
    oth = t[(t // 128) % 2 != p]
    perm = np.empty(T, np.int64)
    pos = np.arange(OWN)
    perm[512 * (pos // 256) + pos % 256] = own
    perm[512 * (pos // 256) + 256 + pos % 256] = oth
    return perm


def _mask_for_parity(p):
    si = np.arange(P)[:, None]
    ti = np.arange(512)[None, :]
    rel_q = np.where(
        ti < 256,
        256 * (ti // 128) + 128 * p + ti % 128,
        256 * ((ti - 256) // 128) + 128 * (1 - p) + (ti - 256) % 128)
    m = np.empty((P, 1024), np.float32)
    for e in range(2):
        key_rel = 256 * e + 128 * p + si
        m[:, 512 * e:512 * (e + 1)] = np.where(key_rel <= rel_q, 1.0, 0.0)
    return m


def make_in_maps(x, Wq, Wk, Wv):
    bf = ml_dtypes.bfloat16
    wq_in = np.zeros((P, CC * 64), np.float32)
    wkv_in = np.zeros((P, CC * 128), np.float32)
    for ci in range(CC):
        wq_in[:, 64 * ci:64 * (ci + 1)] = Wq[P * ci:P * (ci + 1), :]
        wkv_in[:, 128 * ci:128 * ci + 64] = Wk[P * ci:P * (ci + 1), :]
        wkv_in[:, 128 * ci + 64:128 * (ci + 1)] = Wv[P * ci:P * (ci + 1), :]
    in_maps = []
    perms = [_perm_for_parity(0), _perm_for_parity(1)]
    msks = [_mask_for_parity(0), _mask_for_parity(1)]
    for c in range(NCORES):
        b, p = c // 2, c % 2
        xb = np.asarray(x[b], dtype=np.float32)
        xT_in = np.ascontiguousarray(xb[perms[p]].T).astype(bf)
        wmk_in = np.concatenate([wq_in, wkv_in, msks[p]], axis=1).astype(bf)
        in_maps.append({"xT": xT_in, "wmk": wmk_in})
    return in_maps


def combine(results, B=4):
    perms = [_perm_for_parity(0), _perm_for_parity(1)]
    out = np.zeros((B, T, H), np.float32)
    num = np.zeros((T, H), np.float32)
    den = np.zeros((T,), np.float32)
    for b in range(B):
        num[:] = 0.0
        den[:] = 0.0
        for p in range(2):
            o = results[2 * b + p]["outp"].astype(np.float32)
            cols = perms[p].reshape(NJ, 512)
            for j in range(NJ):
                num[cols[j]] += o[j, :64, :].T
                den[cols[j]] += o[j, 64, :]
        out[b] = num / den[:, None]
    return out


def kernel(x, Wq, Wk, Wv, **run_kwargs):
    nc = get_nc()
    in_maps = make_in_maps(x, Wq, Wk, Wv)
    res = bass_utils.run_bass_kernel_spmd(nc, in_maps,
                                          list(range(NCORES)), **run_kwargs)
    out = combine(res.results, B=x.shape[0])
    if run_kwargs:
        kernel.last_results = res
    return out


# revision 5
# speedup vs baseline: 1.0094x; 1.0025x over previous
"""Single-head causal attention (B=4, T=4096, C=768, H=64) on 8 NeuronCores.

Sharding: 2 cores per batch; keys split between the pair by interleaved
128-row blocks.  Host permutes token columns per core (own/other groups of
256) so the device program is parity-independent; host adds the two cores'
unnormalized partials and normalizes.

The Tile framework preserves per-engine emission order, so this version is
explicitly software-pipelined with precise-dependency filler queues:
 - kv-block matmuls ("kvN") and the v'-tile builds ("kvNv") are separate
   queues; q-projections ("qN") likewise.  A queue is force-drained right
   before the first instruction that needs its result; otherwise units are
   popped between score pairs as PE filler.
 - each tile's diagonal pair is computed FIRST (its Pool mask-multiply gets
   a whole tile of slack) and its PV + output copy are DEFERRED into the
   next tile.

Math per core (bf16 projections/scores; fp8 softmax weights; v split into
hi+lo fp8, PV via fp8 DoubleRow matmuls; denominator via ones-column).
"""

import sys

for _p in ("/opt/trn_rl_repo",):
    if _p not in sys.path:
        sys.path.insert(0, _p)

import math
from collections import OrderedDict, deque
import numpy as np
import ml_dtypes

import concourse.bass as bass
import concourse.mybir as mybir
import concourse.tile as tile
from concourse import bacc
from concourse import bass_utils
from concourse.masks import make_identity

BF16 = mybir.dt.bfloat16
F8 = mybir.dt.float8e4
F32 = mybir.dt.float32

P = 128
T = 4096
C = 768
H = 64
CC = C // P
OWN = T // 2
NJ = T // 512
NCORES = 8
LN8 = math.log(8.0)
NWMK = CC * 64 + CC * 128 + 1024

_NC_CACHE = {}


def _build_nc():
    nc = bacc.Bacc("TRN2", target_bir_lowering=False, debug=False,
                   num_devices=NCORES)

    xT = nc.dram_tensor("xT", [C, T], BF16, kind="ExternalInput")
    wmk = nc.dram_tensor("wmk", [P, NWMK], BF16, kind="ExternalInput")
    outp = nc.dram_tensor("outp", [NJ, 65, 512], F32, kind="ExternalOutput")

    with tile.TileContext(nc) as tc:
        with (
            tc.tile_pool(name="const", bufs=1) as cst,
            tc.tile_pool(name="big", bufs=1) as big,
            tc.tile_pool(name="pps", bufs=1, space="PSUM") as pps,
            tc.tile_pool(name="sps", bufs=2, space="PSUM") as sps_pool,
            tc.tile_pool(name="ops", bufs=1, space="PSUM") as ops_pool,
            tc.tile_pool(name="qpp", bufs=1, space="PSUM") as qpp,
            tc.tile_pool(name="wt", bufs=8) as wt_pool,
            tc.tile_pool(name="osb", bufs=4) as osb_pool,
        ):
            bias_t = cst.tile([P, 1], F32)
            nc.vector.memset(bias_t[:], -LN8)
            warm = cst.tile([P, 1], F8)
            nc.scalar.activation(warm[:], bias_t[:],
                                 mybir.ActivationFunctionType.Exp,
                                 scale=0.125, bias=bias_t[:])
            ident = cst.tile([P, P], BF16)
            make_identity(nc, ident[:])
            # p-state warmers: dependency-free junk matmuls fill the
            # DMA-wait startup gap so the PE ramp (3us continuous busy ->
            # full clock) is satisfied before real work arrives.
            jnk = pps.tile([64, 32], F32, tag="vp", name="jnk")
            for _ in range(150):
                nc.tensor.matmul(jnk[:], ident[:, 0:64], ident[:, 0:32],
                                 start=True, stop=True)
            wmk_sb = cst.tile([P, NWMK], BF16)
            nc.sync.dma_start(wmk_sb[:, 0:CC * 192], wmk[:, 0:CC * 192])
            nc.sync.dma_start(wmk_sb[:, CC * 192:], wmk[:, CC * 192:])

            def wq_ap(ci):
                return wmk_sb[:, 64 * ci:64 * (ci + 1)]

            def wkv_ap(ci):
                return wmk_sb[:, CC * 64 + 128 * ci:CC * 64 + 128 * (ci + 1)]

            msk_ap = wmk_sb[:, CC * 192:CC * 192 + 1024]

            xts = []
            for ci in range(CC):
                t_ = big.tile([P, T], BF16, tag=f"xt{ci}")
                xts.append(t_)
            pieces = [(0, 1024), (1024, 2048), (2048, 3072), (3072, 4096)]
            for lo, hi in pieces:
                for ci in range(CC):
                    nc.sync.dma_start(xts[ci][:, lo:hi],
                                      xT[P * ci:P * (ci + 1), lo:hi])

            vhi, vlo = [], []
            for pb in range(NJ):
                th = big.tile([P, 256], F8, tag=f"vhi{pb}")
                nc.gpsimd.memset(th[:], 0.0)
                nc.gpsimd.memset(th[:, 64:65], 1.0)
                nc.gpsimd.memset(th[:, 192:193], 1.0)
                vhi.append(th)
                tl = big.tile([P, 256], F8, tag=f"vlo{pb}")
                nc.gpsimd.memset(tl[:], 0.0)
                vlo.append(tl)

            kvts = [big.tile([P, 512], BF16, tag=f"kvT{tb}",
                             name=f"kvT{tb}") for tb in range(4)]
            qts = [big.tile([64, 512], BF16, tag=f"qT{j}",
                            name=f"qT{j}") for j in range(NJ)]

            # ---- precise-dependency filler queues ----
            queues = OrderedDict()   # name -> deque of closures

            def q_add(name, fn):
                queues.setdefault(name, deque()).append(fn)

            def require(name):
                q = queues.get(name)
                if not q:
                    return
                while q:
                    q.popleft()()

            def pop_filler(n):
                for _ in range(n):
                    for name, q in queues.items():
                        if q:
                            q.popleft()()
                            break
                    else:
                        return

            def queue_kv(tb):
                ps_box = {}

                def mm(ci):
                    def f():
                        if ci == 0:
                            ps_box["ps"] = pps.tile([P, 512], F32,
                                                    tag="pps", name="ps")
                        src = xts[ci][:].rearrange("p (g c) -> p g c", c=256)
                        nc.tensor.matmul(
                            ps_box["ps"][:], wkv_ap(ci),
                            src[:, 4 * tb:4 * tb + 4:2, :],
                            start=(ci == 0), stop=(ci == CC - 1))
                    return f

                def kvt_copy():
                    nc.vector.tensor_copy(kvts[tb][:], ps_box["ps"][:])

                for ci in range(CC):
                    q_add(f"kv{tb}", mm(ci))
                q_add(f"kv{tb}", kvt_copy)

                def transp(i):
                    def f():
                        if i == 0:
                            ps_box["vp"] = pps.tile([P, 256], BF16,
                                                    tag="vp", name="vp")
                        nc.tensor.transpose(
                            ps_box["vp"][:, 64 * i:64 * (i + 1)],
                            kvts[tb][64:128, 128 * i:128 * (i + 1)],
                            ident[64:128, 64:128])
                    return f

                def vcopy(i):
                    def f():
                        g = 4 * tb + i
                        pb, s = g // 2, g % 2
                        dst_h = vhi[pb][:, 128 * s:128 * s + 64]
                        dst_l = vlo[pb][:, 128 * s:128 * s + 64]
                        vp = ps_box["vp"]
                        nc.vector.tensor_copy(dst_h, vp[:, 64 * i:64 * (i + 1)])
                        nc.vector.tensor_sub(dst_l, vp[:, 64 * i:64 * (i + 1)],
                                             dst_h)
                    return f

                for i in range(4):
                    q_add(f"kv{tb}v", transp(i))
                for i in range(4):
                    q_add(f"kv{tb}v", vcopy(i))

            def queue_q(j):
                qp_box = {}

                def mm(ci):
                    def f():
                        if ci == 0:
                            qp_box["qp"] = qpp.tile([64, 512], F32,
                                                    tag="qp", name="qp")
                        nc.tensor.matmul(
                            qp_box["qp"][:], wq_ap(ci),
                            xts[ci][:, 512 * j:512 * (j + 1)],
                            start=(ci == 0), stop=(ci == CC - 1))
                    return f

                def qt_copy():
                    nc.vector.tensor_copy(qts[j][:], qp_box["qp"][:])

                for ci in range(CC):
                    q_add(f"q{j}", mm(ci))
                q_add(f"q{j}", qt_copy)

            def scores_pair(sp, pb, j):
                for i in range(2):
                    g = 2 * pb + i
                    nc.tensor.matmul(
                        sp[:, 512 * i:512 * (i + 1)],
                        kvts[g // 4][0:64, 128 * (g % 4):128 * (g % 4 + 1)],
                        qts[j][:], start=True, stop=True)

            def dr_pair(ops, pb, wt, start, stop):
                require(f"kv{pb // 2}v")
                wt3 = wt[:].rearrange("p (two n) -> p two n", two=2)
                nc.tensor.matmul(
                    ops[:], vhi[pb][:].rearrange("p (two m) -> p two m", two=2),
                    wt3, start=start, stop=False,
                    perf_mode=mybir.MatmulPerfMode.DoubleRow)
                nc.tensor.matmul(
                    ops[:], vlo[pb][:].rearrange("p (two m) -> p two m", two=2),
                    wt3, start=False, stop=stop,
                    perf_mode=mybir.MatmulPerfMode.DoubleRow)

            # pending diagonal PV from the previous tile: (wtd, j, ops, start)
            state = {"diag": None}

            def flush_diag():
                pend = state["diag"]
                if pend is None:
                    return
                wtd, jj, ops_prev, start = pend
                state["diag"] = None
                dr_pair(ops_prev, jj, wtd, start, True)
                osb = osb_pool.tile([65, 512], F32, tag="osb")
                nc.vector.tensor_copy(osb[:], ops_prev[0:65, :])
                nc.sync.dma_start(outp[jj], osb[:])

            # queue all projection work up front (popped as filler in order;
            # force-drained by require() right before first use)
            queue_kv(0)
            queue_q(0)
            queue_q(1)

            for j in range(NJ):
                if j % 2 == 1 and (j + 1) // 2 < 4:
                    queue_kv((j + 1) // 2)
                if j + 2 < NJ:
                    queue_q(j + 2)

                require(f"kv{(2 * j + 1) // 4}")
                require(f"q{j}")

                ops = ops_pool.tile([P, 512], F32, tag="ops")
                # diagonal pair first; the odd chunk only needs query cols
                # {128:256, 384:512} (union of both parities' unmasked cols)
                spd = sps_pool.tile([P, 1024], F32, tag="sps")
                g0 = 2 * j
                nc.tensor.matmul(
                    spd[:, 0:512],
                    kvts[g0 // 4][0:64, 128 * (g0 % 4):128 * (g0 % 4 + 1)],
                    qts[j][:], start=True, stop=True)
                g1 = 2 * j + 1
                kslc = kvts[g1 // 4][0:64, 128 * (g1 % 4):128 * (g1 % 4 + 1)]
                nc.tensor.matmul(spd[:, 640:768], kslc, qts[j][:, 128:256],
                                 start=True, stop=True)
                nc.tensor.matmul(spd[:, 896:1024], kslc, qts[j][:, 384:512],
                                 start=True, stop=True)
                wtd = wt_pool.tile([P, 1024], F8, tag="wtd")
                nc.scalar.activation(
                    wtd[:, 0:512], spd[:, 0:512],
                    mybir.ActivationFunctionType.Exp,
                    scale=0.125, bias=bias_t[:])
                spd3 = spd[:].rearrange("p (g c) -> p g c", c=128)
                wtd3 = wtd[:].rearrange("p (g c) -> p g c", c=128)
                nc.scalar.activation(
                    wtd3[:, 5:8:2, :], spd3[:, 5:8:2, :],
                    mybir.ActivationFunctionType.Exp,
                    scale=0.125, bias=bias_t[:])
                nc.gpsimd.memset(wtd3[:, 4:7:2, :], 0.0)
                nc.gpsimd.tensor_mul(wtd[:], wtd[:], msk_ap)
                pop_filler(2)

                pend = []
                for pb in range(j):
                    sp = sps_pool.tile([P, 1024], F32, tag="sps")
                    scores_pair(sp, pb, j)
                    wt = wt_pool.tile([P, 1024], F8, tag="wt")
                    nc.scalar.activation(
                        wt[:], sp[:], mybir.ActivationFunctionType.Exp,
                        scale=0.125, bias=bias_t[:])
                    if pb == 0:
                        flush_diag()     # previous tile's diagonal PV + out
                    pend.append((wt, pb))
                    if len(pend) > 2:
                        w0, p0 = pend.pop(0)
                        dr_pair(ops, p0, w0, p0 == 0, False)
                    pop_filler(3)
                for w0, p0 in pend:
                    dr_pair(ops, p0, w0, p0 == 0, False)
                pend = []
                flush_diag()             # no-op except after tile 0
                state["diag"] = (wtd, j, ops, j == 0)
            flush_diag()

    nc.compile()
    return nc


def get_nc():
    if "nc" not in _NC_CACHE:
        _NC_CACHE["nc"] = _build_nc()
    return _NC_CACHE["nc"]


def _perm_for_parity(p):
    t = np.arange(T)
    own = t[(t // 128) % 2 == p]
    oth = t[(t // 128) % 2 != p]
    perm = np.empty(T, np.int64)
    pos = np.arange(OWN)
    perm[512 * (pos // 256) + pos % 256] = own
    perm[512 * (pos // 256) + 256 + pos % 256] = oth
    return perm


def _mask_for_parity(p):
    si = np.arange(P)[:, None]
    ti = np.arange(512)[None, :]
    rel_q = np.where(
        ti < 256,
        256 * (ti // 128) + 128 * p + ti % 128,
        256 * ((ti - 256) // 128) + 128 * (1 - p) + (ti - 256) % 128)
    m = np.empty((P, 1024), np.float32)
    for e in range(2):
        key_rel = 256 * e + 128 * p + si
        m[:, 512 * e:512 * (e + 1)] = np.where(key_rel <= rel_q, 1.0, 0.0)
    return m


def make_in_maps(x, Wq, Wk, Wv):
    bf = ml_dtypes.bfloat16
    wq_in = np.zeros((P, CC * 64), np.float32)
    wkv_in = np.zeros((P, CC * 128), np.float32)
    for ci in range(CC):
        wq_in[:, 64 * ci:64 * (ci + 1)] = Wq[P * ci:P * (ci + 1), :]
        wkv_in[:, 128 * ci:128 * ci + 64] = Wk[P * ci:P * (ci + 1), :]
        wkv_in[:, 128 * ci + 64:128 * (ci + 1)] = Wv[P * ci:P * (ci + 1), :]
    in_maps = []
    perms = [_perm_for_parity(0), _perm_for_parity(1)]
    msks = [_mask_for_parity(0), _mask_for_parity(1)]
    for c in range(NCORES):
        b, p = c // 2, c % 2
        xb = np.asarray(x[b], dtype=np.float32)
        xT_in = np.ascontiguousarray(xb[perms[p]].T).astype(bf)
        wmk_in = np.concatenate([wq_in, wkv_in, msks[p]], axis=1).astype(bf)
        in_maps.append({"xT": xT_in, "wmk": wmk_in})
    return in_maps


def combine(results, B=4):
    perms = [_perm_for_parity(0), _perm_for_parity(1)]
    out = np.zeros((B, T, H), np.float32)
    num = np.zeros((T, H), np.float32)
    den = np.zeros((T,), np.float32)
    for b in range(B):
        num[:] = 0.0
        den[:] = 0.0
        for p in range(2):
            o = results[2 * b + p]["outp"].astype(np.float32)
            cols = perms[p].reshape(NJ, 512)
            for j in range(NJ):
                num[cols[j]] += o[j, :64, :].T
                den[cols[j]] += o[j, 64, :]
        out[b] = num / den[:, None]
    return out


def kernel(x, Wq, Wk, Wv, **run_kwargs):
    nc = get_nc()
    in_maps = make_in_maps(x, Wq, Wk, Wv)
    res = bass_utils.run_bass_kernel_spmd(nc, in_maps,
                                          list(range(NCORES)), **run_kwargs)
    out = combine(res.results, B=x.shape[0])
    if run_kwargs:
        kernel.last_results = res
    return out
